# revision 1
# baseline (speedup 1.0000x reference)
"""nn_AttentionOpt on 8 Trainium2 NeuronCores.

Data-parallel over batch N=8: one batch element per core. Each core computes,
for its (C=1024, L=1024) slice:

    x = seq^T                       (L, C)
    Q/K = relu(x @ W^T + b)         kept transposed:  Q^T, K^T  (C, L)  bf16
    V   = relu(x @ Wv^T + bv) * m   kept straight:    V (L, C), augmented with
                                    64 copies of the key mask as extra columns
    per head h (dh=64):
      S^T  = K_h Q_h^T / 8          ([k, q] layout, PE, 2 heads row-tiled)
      E^T  = exp(S^T)               (ACT, no max-subtraction needed: S <= ~3)
      P    = [m*ones_64 | V_h]^T E^T   -> rows 0:64 = softmax denominator,
                                         rows 64:128 = unnormalized out^T
      y_h  = P[64:128] * approx_recip(P[0:64])
    LayerNorm over C (partition dim) via f32r ones-matmul stats, then
    out = ((y - u) * ln_w * rstd * m + ln_b * m)   (post-mask only)

Math identities used (validated vs the reference to ~5e-6 rel):
  * key mask folded into V rows + denominator (no masking of scores needed)
  * query mask + pre-LN mask both subsumed by the final post-LN mask
  * softmax needs no max-subtraction (scores in [0, 2.7])
"""
import sys

if "/opt/trn_rl_repo" not in sys.path:
    sys.path.insert(0, "/opt/trn_rl_repo")

from contextlib import ExitStack

import numpy as np

import concourse.bass as bass
import concourse.tile as tile
from concourse import bacc, mybir
from concourse.bass_utils import run_bass_kernel_spmd

f32 = mybir.dt.float32
f32r = mybir.dt.float32r
bf16 = mybir.dt.bfloat16
AF = mybir.ActivationFunctionType
ALU = mybir.AluOpType

N_CORES = 8
C = 1024
L = 1024
H = 16
DH = 64
P = 128
NCH = C // P          # 8 chunks of channels
NLC = L // P          # 8 chunks of positions (key chunks)
FD = 512              # matmul moving free dim (one PSUM bank of f32)
NQ = L // FD          # 2 query halves
EPS = 1e-5
SCALE = 1.0 / 8.0     # 1/sqrt(DH)

_BUILT = None
LAST_RESULTS = None
DEBUG_TAPS = False  # when True, adds debug ExternalOutputs for HW-vs-sim bisect


def _bcast_ap(ap, n):
    """Prepend a [0, n] partition-broadcast dim to an AP (DRAM source)."""
    return bass.AP(tensor=ap.tensor, offset=ap.offset, ap=[[0, n]] + list(ap.ap))


def _emit(tc, io):
    nc = tc.nc
    seq, maskf = io["seq"], io["maskf"]
    ws = {k: io[k] for k in ("wq", "wk", "wv")}
    bs = {k: io[k] for k in ("bq", "bk", "bv")}
    lnw, lnb, out = io["ln_w"], io["ln_b"], io["out"]

    with ExitStack() as ctx:
        persist = ctx.enter_context(tc.tile_pool(name="persist", bufs=1))
        dram = ctx.enter_context(tc.tile_pool(name="dram", bufs=1, space="DRAM"))
        mm = ctx.enter_context(tc.tile_pool(name="mm", bufs=2, space="PSUM"))
        stats = ctx.enter_context(tc.tile_pool(name="stats", bufs=1, space="PSUM"))

        # ---- constants ----------------------------------------------------
        bq_sb = persist.tile([P, NCH], f32, tag="bq")
        bk_sb = persist.tile([P, NCH], f32, tag="bk")
        nc.sync.dma_start(out=bq_sb[:], in_=bs["bq"].rearrange("(a p) -> p a", p=P))
        nc.sync.dma_start(out=bk_sb[:], in_=bs["bk"].rearrange("(a p) -> p a", p=P))
        m_l = persist.tile([P, NLC], f32, tag="ml")
        nc.sync.dma_start(out=m_l[:], in_=maskf.rearrange("(a p) -> p a", p=P))
        lnw_sb = persist.tile([P, NCH], f32, tag="lnw")
        lnb_sb = persist.tile([P, NCH], f32, tag="lnb")
        nc.sync.dma_start(out=lnw_sb[:], in_=lnw.rearrange("(a p) -> p a", p=P))
        nc.sync.dma_start(out=lnb_sb[:], in_=lnb.rearrange("(a p) -> p a", p=P))
        ones_f = persist.tile([P, 1], f32, tag="onesf")
        nc.vector.memset(ones_f[:], 1.0)
        ones_col = persist.tile([P, 1], f32r, tag="ones")
        nc.vector.tensor_copy(ones_col[:], ones_f[:])

        # ---- persistent big tensors --------------------------------------
        qT = persist.tile([P, NCH, L], bf16, tag="qT")
        kT = persist.tile([P, NCH, L], bf16, tag="kT")
        v_aug = persist.tile([P, NLC, H, P], bf16, tag="vaug")
        y_sb = persist.tile([P, NCH, L], f32r, tag="y")

        # mask columns 64:128 of v_aug (denominator rows of the AV matmul)
        ml_b = bass.AP(tensor=m_l.tensor, offset=m_l.offset,
                       ap=list(m_l[:].ap[:2]) + [[0, H], [0, DH]])
        nc.vector.tensor_copy(v_aug[:, :, :, 0:DH], ml_b)

        # channel-sum accumulator (alive across the whole attention phase)
        ps_sy = stats.tile([1, L], f32, tag="sy")

        zero_col = persist.tile([P, 1], f32, tag="zero")
        nc.vector.memset(zero_col[:], 0.0)
        ones_row_f = persist.tile([1, P], f32, tag="onesrowf")
        nc.vector.memset(ones_row_f[:], 1.0)
        ones_row = persist.tile([1, P], f32r, tag="onesrow")
        nc.vector.tensor_copy(ones_row[:], ones_row_f[:])

        with tc.tile_pool(name="projw", bufs=1) as projw:
            # ---- seq -> bf16 SBUF; W -> bf16 DRAM staging ----------------
            seq_bf = projw.tile([P, NCH, L], bf16, tag="seq")
            w_bf = {}
            for name in ("wq", "wk", "wv"):
                w_bf[name] = dram.tile([C, C], bf16, tag=f"{name}bf",
                                       name=f"{name}bf")
            # wq cast first: the first projection chunk needs only wqT + seq
            nc.gpsimd.dma_start(out=w_bf["wq"][:], in_=ws["wq"])
            nc.gpsimd.dma_start(
                out=seq_bf[:], in_=seq.rearrange("(cc p) l -> p cc l", p=P))
            nc.gpsimd.dma_start(out=w_bf["wk"][:], in_=ws["wk"])
            nc.gpsimd.dma_start(out=w_bf["wv"][:], in_=ws["wv"])
            with tc.tile_pool(name="wt", bufs=4) as wtpool:

                def qk_chunk(dc):
                    for tgt, wn, b_sb in ((qT, "wq", bq_sb), (kT, "wk", bk_sb)):
                        # stream this chunk's transposed weights: 8 [128,128]
                        # DMA transposes into a small recycled tile
                        wt = wtpool.tile([P, NCH, P], bf16, tag="wt",
                                         name=f"wt_{wn}{dc}")
                        for cc in range(NCH):
                            nc.sync.dma_start(
                                out=wt[:, cc, :],
                                in_=w_bf[wn][dc * P:(dc + 1) * P,
                                             cc * P:(cc + 1) * P],
                                transpose=True)
                        for lh in range(NQ):
                            ps = mm.tile([P, FD], f32, tag="mm")
                            for cc in range(NCH):
                                nc.tensor.matmul(
                                    ps[:],
                                    lhsT=wt[:, cc, :],
                                    rhs=seq_bf[:, cc, lh * FD:(lh + 1) * FD],
                                    start=(cc == 0), stop=(cc == NCH - 1))
                            # relu(x + b) on the DVE: (ps add b) max 0
                            nc.vector.scalar_tensor_tensor(
                                out=tgt[:, dc, lh * FD:(lh + 1) * FD],
                                in0=ps[:], scalar=b_sb[:, dc:dc + 1],
                                in1=zero_col[:].to_broadcast((P, FD)),
                                op0=ALU.add, op1=ALU.max)

                with tc.tile_pool(name="eT", bufs=3) as epool, \
                     tc.tile_pool(name="sc", bufs=2, space="PSUM") as scp:

                    def alloc_pair(hp):
                        return {h: epool.tile([P, NLC, L], bf16, tag="eT",
                                              name=f"eT_h{h}")
                                for h in (2 * hp, 2 * hp + 1)}

                    def scores_pair(hp, eTs):
                        # scores + exp: the two heads of a pair live at row
                        # groups 0:63 / 64:127, so ALTERNATE them in the PE
                        # queue (a-qh0, b-qh0, a-qh1, b-qh1) — adjacent
                        # matmuls in different row groups run concurrently
                        # on the 16x 32x32 sub-arrays
                        for kc in range(NLC):
                            pss = {}
                            for off, h in ((0, 2 * hp), (DH, 2 * hp + 1)):
                                pss[h] = scp.tile([P, L], f32, tag="sc",
                                                  name=f"sc_h{h}")
                            for qh in range(NQ):
                                for off, h in ((0, 2 * hp), (DH, 2 * hp + 1)):
                                    nc.tensor.matmul(
                                        pss[h][:, qh * FD:(qh + 1) * FD],
                                        lhsT=kT[off:off + DH, hp, kc * P:(kc + 1) * P],
                                        rhs=qT[off:off + DH, hp, qh * FD:(qh + 1) * FD],
                                        start=True, stop=True)
                            for off, h in ((0, 2 * hp), (DH, 2 * hp + 1)):
                                nc.scalar.activation(
                                    eTs[h][:, kc, :], pss[h][:], AF.Exp,
                                    scale=SCALE)

                    # pair-0 scores go before the V projection so the ACT engine
                    # starts the exp stream as early as possible
                    qk_chunk(0)
                    eTs_cur = alloc_pair(0)
                    scores_pair(0, eTs_cur)

                    # ---- V projection (wvT + temps die right after) --------------
                    with tc.tile_pool(name="vw", bufs=1) as vw:
                        wvT = vw.tile([P, NCH, C], bf16, tag="wvT")
                        bv_row = vw.tile([1, C], bf16, tag="bvrow")
                        nc.gpsimd.dma_start(
                            out=bv_row[:], in_=bs["bv"].rearrange("(a c) -> a c", a=1))
                        ones1_f = vw.tile([1, P], f32, tag="ones1f")
                        nc.vector.memset(ones1_f[:], 1.0)
                        ones1 = vw.tile([1, P], bf16, tag="ones1")
                        nc.vector.tensor_copy(ones1[:], ones1_f[:])
                        for cc in range(NCH):
                            nc.sync.dma_start(out=wvT[:, cc, :],
                                              in_=w_bf["wv"][:, cc * P:(cc + 1) * P],
                                              transpose=True)
                        for lc in range(NLC):
                            for dh2 in range(NQ):
                                ps = mm.tile([P, FD], f32, tag="mm")
                                # bias via a K=1 ones matmul into the accumulator
                                nc.tensor.matmul(
                                    ps[:], lhsT=ones1[:],
                                    rhs=bv_row[:, dh2 * FD:(dh2 + 1) * FD],
                                    start=True, stop=False)
                                for cc in range(NCH):
                                    nc.tensor.matmul(
                                        ps[:],
                                        lhsT=seq_bf[:, cc, lc * P:(lc + 1) * P],
                                        rhs=wvT[:, cc, dh2 * FD:(dh2 + 1) * FD],
                                        start=False, stop=(cc == NCH - 1))
                                # relu + query-position mask on the DVE:
                                # (ps max 0) * m[l]  (m broadcast along free dim)
                                nc.vector.scalar_tensor_tensor(
                                    out=v_aug[:, lc,
                                              dh2 * (H // 2):(dh2 + 1) * (H // 2), DH:P],
                                    in0=ps[:], scalar=0.0,
                                    in1=m_l[:, lc:lc + 1].to_broadcast((P, FD)),
                                    op0=ALU.max, op1=ALU.mult)

                    with tc.tile_pool(name="att", bufs=2) as att:
                        for hp in range(H // 2):
                            heads = ((0, 2 * hp), (DH, 2 * hp + 1))
                            eTs = eTs_cur
                            if hp > 0:
                                scores_pair(hp, eTs)
                            if hp + 1 < NCH:
                                qk_chunk(hp + 1)
                            if hp + 1 < H // 2:
                                eTs_cur = alloc_pair(hp + 1)
                            # attention-value matmul + normalization
                            for off, h in heads:
                                for qh in range(NQ):
                                    ps = mm.tile([P, FD], f32, tag="mm")
                                    for kc in range(NLC):
                                        nc.tensor.matmul(
                                            ps[:],
                                            lhsT=v_aug[:, kc, h, :],
                                            rhs=eTs[h][:, kc, qh * FD:(qh + 1) * FD],
                                            start=(kc == 0), stop=(kc == NLC - 1))
                                    if DEBUG_TAPS and h == 0 and qh == 0:
                                        dbg_ps = att.tile([P, FD], f32, tag="dbgps")
                                        nc.vector.tensor_copy(dbg_ps[:], ps[:])
                                        od = nc.dram_tensor("dbg_pso", [P, FD], f32,
                                                            kind="ExternalOutput").ap()
                                        nc.sync.dma_start(out=od, in_=dbg_ps[:])
                                        od2 = nc.dram_tensor("dbg_eT0", [P, NLC * L],
                                                             bf16,
                                                             kind="ExternalOutput").ap()
                                        nc.sync.dma_start(
                                            out=od2,
                                            in_=eTs[h][:].rearrange("p a b -> p (a b)"))
                                    # denominators sit at PSUM partition 0,
                                    # where the custom-DVE op is HW-correct
                                    rcp = att.tile([DH, FD], f32, tag="rcp")
                                    nc.vector.reciprocal_approx_fast(
                                        out=rcp[:], in_=ps[0:DH, :])
                                    nc.vector.tensor_mul(
                                        y_sb[off:off + DH, hp, qh * FD:(qh + 1) * FD],
                                        ps[DH:P, :], rcp[:])
                            # channel-sum statistics for the finished channel chunk
                            for qh in range(NQ):
                                nc.tensor.matmul(
                                    ps_sy[:, qh * FD:(qh + 1) * FD],
                                    lhsT=ones_col[:],
                                    rhs=y_sb[:, hp, qh * FD:(qh + 1) * FD],
                                    start=(hp == 0), stop=(hp == NCH - 1))



        # ---- LayerNorm tail ----------------------------------------------
        # u = mean_c(y); t1 = y - u; var = mean_c(t1^2)  (two-pass, avoids
        # the E[y^2]-u^2 cancellation: y has a large common mode)
        with tc.tile_pool(name="rows", bufs=1) as rows, \
             tc.tile_pool(name="tailps", bufs=1, space="PSUM") as tailps, \
             tc.tile_pool(name="norm", bufs=4) as norm:
            m_rep = rows.tile([P, L], f32, tag="mrep")
            nc.gpsimd.dma_start(out=m_rep[:], in_=_bcast_ap(maskf, P))
            u_row = rows.tile([1, L], f32r, tag="u")
            nc.scalar.mul(u_row[:], ps_sy[:], 1.0 / C)
            # broadcast u across partitions with K=1 matmuls (no DRAM trip)
            u_rep = tailps.tile([P, L], f32, tag="urep")
            for qh in range(NQ):
                nc.tensor.matmul(u_rep[:, qh * FD:(qh + 1) * FD],
                                 lhsT=ones_row[:],
                                 rhs=u_row[:, qh * FD:(qh + 1) * FD],
                                 start=True, stop=True)

            # two-pass variance: t1 = y - u on gpsimd (kept for the final
            # normalize), square on ACT, ones-matmul partition-reduce on PE
            ps_var = stats.tile([1, L], f32, tag="sy", name="ps_var")
            t1 = rows.tile([P, NCH, L], f32, tag="t1")
            for cc in range(NCH):
                nc.vector.tensor_sub(t1[:, cc, :], y_sb[:, cc, :], u_rep[:])
                t1sq = norm.tile([P, L], f32r, tag="t1sq")
                nc.scalar.square(t1sq[:], t1[:, cc, :])
                for qh in range(NQ):
                    nc.tensor.matmul(
                        ps_var[0:1, qh * FD:(qh + 1) * FD],
                        lhsT=ones_col[:],
                        rhs=t1sq[:, qh * FD:(qh + 1) * FD],
                        start=(cc == 0), stop=(cc == NCH - 1))
            var_row = rows.tile([1, L], f32, tag="var")
            nc.scalar.mul(var_row[:], ps_var[:], 1.0 / C)
            eps_col = rows.tile([1, 1], f32, tag="eps")
            nc.vector.memset(eps_col[:], EPS)
            ln_row = rows.tile([1, L], f32, tag="lnr")
            nc.scalar.activation(ln_row[:], var_row[:], AF.Ln,
                                 bias=eps_col[:, 0:1])
            rstd_row = rows.tile([1, L], f32r, tag="rstd")
            nc.scalar.activation(rstd_row[:], ln_row[:], AF.Exp, scale=-0.5)
            rm_rep = tailps.tile([P, L], f32, tag="rmrep")
            for qh in range(NQ):
                nc.tensor.matmul(rm_rep[:, qh * FD:(qh + 1) * FD],
                                 lhsT=ones_row[:],
                                 rhs=rstd_row[:, qh * FD:(qh + 1) * FD],
                                 start=True, stop=True)

            if DEBUG_TAPS:
                nc_ = tc.nc
                taps = {
                    "dbg_seq": (seq_bf[:, 0, :], bf16),
                    "dbg_wvT": (wvT[:, 0, :], bf16),
                    "dbg_qT": (qT[:, 0, :], bf16),
                    "dbg_kT": (kT[:, 0, :], bf16),
                    "dbg_vaug": (v_aug[:, 0, :, :].rearrange("p a b -> p (a b)"),
                                 bf16),
                    "dbg_y": (y_sb[:, 0, :].bitcast(f32), f32),
                    "dbg_t1": (t1[:, 0, :], f32),
                    "dbg_urep": (u_rep[:], f32),
                    "dbg_rmrep": (rm_rep[:], f32),
                    "dbg_mrep": (m_rep[:], f32),
                    "dbg_bvrep": (bv_rep[:], f32),
                    "dbg_ml": (m_l[:], f32),
                }
                for tname, (src, dt_) in taps.items():
                    shp = list(src.shape)
                    od = nc_.dram_tensor(tname, shp, dt_,
                                         kind="ExternalOutput").ap()
                    nc_.sync.dma_start(out=od, in_=src)

            out_r = out.rearrange("(cc p) l -> p cc l", p=P)
            for cc in range(NCH):
                t2 = norm.tile([P, L], f32, tag="t2")
                nc.vector.scalar_tensor_tensor(
                    out=t2[:], in0=t1[:, cc, :], scalar=lnw_sb[:, cc:cc + 1],
                    in1=rm_rep[:], op0=ALU.mult, op1=ALU.mult)
                t3 = norm.tile([P, L], f32, tag="t3")
                nc.vector.scalar_tensor_tensor(
                    out=t3[:], in0=t2[:], scalar=lnb_sb[:, cc:cc + 1],
                    in1=m_rep[:], op0=ALU.add, op1=ALU.mult)
                nc.sync.dma_start(out=out_r[:, cc, :], in_=t3[:])


def _pin_act_table(nc):
    """Make every activation resolve to the one table that contains all the
    functions this kernel uses (Exp, Ln, Relu, Copy, Identity, Square), so
    the compiler emits a single LoadActFuncSet instead of thrashing between
    per-function tables. Only the compile-time chooser is constrained; the
    table ids and runtime act_info.json are untouched."""
    from concourse.hw_specs import get_activation_tables
    keep = "natural_log_exp_and_others"
    try:
        tabs = get_activation_tables(nc.m.arch)
    except Exception:
        return
    if keep not in tabs:
        return
    shared = set(tabs[keep])
    for name, funcs in tabs.items():
        if name != keep:
            funcs -= shared


def build():
    global _BUILT
    if _BUILT is not None:
        return _BUILT
    nc = bacc.Bacc("TRN2", target_bir_lowering=False, debug=False,
                   num_devices=N_CORES)
    _pin_act_table(nc)
    io = {
        "seq": nc.dram_tensor("seq", [C, L], f32, kind="ExternalInput").ap(),
        "maskf": nc.dram_tensor("maskf", [L], f32, kind="ExternalInput").ap(),
        "wq": nc.dram_tensor("wq", [C, C], f32, kind="ExternalInput").ap(),
        "bq": nc.dram_tensor("bq", [C], f32, kind="ExternalInput").ap(),
        "wk": nc.dram_tensor("wk", [C, C], f32, kind="ExternalInput").ap(),
        "bk": nc.dram_tensor("bk", [C], f32, kind="ExternalInput").ap(),
        "wv": nc.dram_tensor("wv", [C, C], f32, kind="ExternalInput").ap(),
        "bv": nc.dram_tensor("bv", [C], f32, kind="ExternalInput").ap(),
        "ln_w": nc.dram_tensor("ln_w", [C], f32, kind="ExternalInput").ap(),
        "ln_b": nc.dram_tensor("ln_b", [C], f32, kind="ExternalInput").ap(),
        "out": nc.dram_tensor("out", [C, L], f32, kind="ExternalOutput").ap(),
    }
    with tile.TileContext(nc) as tc:
        _emit(tc, io)
    nc.compile()
    _BUILT = nc
    return nc


def make_in_maps(seq, mask, wq, bq, wk, bk, wv, bv, ln_w, ln_b):
    seq = np.ascontiguousarray(np.asarray(seq, dtype=np.float32))
    mask_f = np.ascontiguousarray(
        np.asarray(mask).astype(np.float32).reshape(N_CORES, L))
    shared = {
        "wq": np.ascontiguousarray(np.asarray(wq, dtype=np.float32)),
        "bq": np.ascontiguousarray(np.asarray(bq, dtype=np.float32)),
        "wk": np.ascontiguousarray(np.asarray(wk, dtype=np.float32)),
        "bk": np.ascontiguousarray(np.asarray(bk, dtype=np.float32)),
        "wv": np.ascontiguousarray(np.asarray(wv, dtype=np.float32)),
        "bv": np.ascontiguousarray(np.asarray(bv, dtype=np.float32)),
        "ln_w": np.ascontiguousarray(np.asarray(ln_w, dtype=np.float32)),
        "ln_b": np.ascontiguousarray(np.asarray(ln_b, dtype=np.float32)),
    }
    return [{"seq": seq[i], "maskf": mask_f[i], **shared} for i in range(N_CORES)]


def kernel(seq, mask, wq, bq, wk, bk, wv, bv, ln_w, ln_b):
    global LAST_RESULTS
    nc = build()
    in_maps = make_in_maps(seq, mask, wq, bq, wk, bk, wv, bv, ln_w, ln_b)
    res = run_bass_kernel_spmd(nc, in_maps, list(range(N_CORES)))
    LAST_RESULTS = res
    return np.stack([res.results[i]["out"] for i in range(N_CORES)], axis=0)



# revision 9
# speedup vs baseline: 1.2250x; 1.2250x over previous
"""nn_AttentionOpt on 8 Trainium2 NeuronCores.

Data-parallel over batch N=8: one batch element per core. Per core
(C=1024 channels, L=1024 positions, H=16 heads, dh=64):

    x = seq^T                        (L, C)
    Q/K = relu(x @ Wq^T + b)         fp8e4, score-sharded layout (see below)
    V   = relu(x @ Wv^T + bv) * m    fp8e4, augmented with the key mask as
                                     64 extra rows (softmax denominator)
    S^T = K_h Q_h^T                  fp8 DoubleRow matmuls ([k, q] layout)
    E   = exp(S^T / (8*SW^2))        ACT, written as fp8e4
    P   = [m*ones | V_h]^T E^T       fp8 DoubleRow: rows 0:64 = denominator,
                                     rows 64:128 = unnormalized out^T
    y_h = P[64:128] * approx_recip(P[0:64])      (f32, kept for LN stats)
    LayerNorm over C via one-pass stats (ones-matmul partition reduce of
    y and y*y during attention), post-mask only.

Performance structure (cost-model driven):
  * All projections run as fp8e4 DoubleRow matmuls (0.5 cycles/row, double
    contraction) with an error-compensating split: x = x_hi + x_lo and
    W = W_hi + W_lo in fp8, computing hi*hi + lo*hi + hi*lo. This gives
    ~bf16-level accuracy at ~2.7x the bf16 matmul rate.
  * Weights are staged host-side: pre-transposed, pre-scaled by SW=32 (to
    keep fp8 values in the normal range), pre-split hi/lo, and (for Wq/Wk)
    row-permuted so the projection PSUM tiles land directly in the
    [32-channel, 2-ksubtile] layout DoubleRow score matmuls need.
  * Scores use single fp8 Q/K (quantization error is suppressed by softmax
    averaging over ~512 unmasked keys); AV uses single fp8 E and V.
  * The exp stream on the ACT engine (~128 x [128,1024] activations) is the
    bottleneck; PE work is interleaved behind it via a background queue.

Scale bookkeeping: Q,K,V all carry SW=32 -> scores carry SW^2 (folded into
the exp scale), y carries SW (cancelled by LayerNorm; EPS scaled by SW^2).
"""
import sys

if "/opt/trn_rl_repo" not in sys.path:
    sys.path.insert(0, "/opt/trn_rl_repo")

from collections import deque
from contextlib import ExitStack

import numpy as np

import concourse.bass as bass
import concourse.tile as tile
from concourse import bacc, mybir
from concourse.bass_utils import run_bass_kernel_spmd

f32 = mybir.dt.float32
f32r = mybir.dt.float32r
bf16 = mybir.dt.bfloat16
f8 = mybir.dt.float8e4
AF = mybir.ActivationFunctionType
ALU = mybir.AluOpType
DR = mybir.MatmulPerfMode.DoubleRow

N_CORES = 8
C = 1024
L = 1024
H = 16
DH = 64
P = 128
NCH = C // P          # 8 chunks of channels
NLC = L // P          # 8 chunks of positions (key chunks)
FD = 512              # matmul moving free dim (one PSUM bank of f32)
NQ = L // FD          # 2 query halves
SW = 32.0             # host-side weight scale (fp8 normal range)
EPS = 1e-5 * SW * SW  # LN epsilon in the SW-scaled domain
SCALE = 1.0 / (8.0 * SW * SW)   # 1/sqrt(dh) folded with 1/SW^2

USE_FP8_SCORES = True

_BUILT = None
LAST_RESULTS = None


def _bcast_ap(ap, n):
    """Prepend a [0, n] partition-broadcast dim to an AP (DRAM source)."""
    return bass.AP(tensor=ap.tensor, offset=ap.offset, ap=[[0, n]] + list(ap.ap))


def _emit(tc, io):
    nc = tc.nc
    out = io["out"]

    with ExitStack() as ctx:
        persist = ctx.enter_context(tc.tile_pool(name="persist", bufs=1))

        # ---- constants / small inputs ------------------------------------
        bq_sb = persist.tile([P, NCH], f32, tag="bq")
        bk_sb = persist.tile([P, NCH], f32, tag="bk")
        nc.gpsimd.dma_start(out=bq_sb[:], in_=io["bq"].rearrange("(a p) -> p a", p=P))
        nc.gpsimd.dma_start(out=bk_sb[:], in_=io["bk"].rearrange("(a p) -> p a", p=P))
        m_l = persist.tile([P, NLC], f32, tag="ml")
        nc.gpsimd.dma_start(out=m_l[:], in_=io["maskf"].rearrange("(a p) -> p a", p=P))
        lnw_sb = persist.tile([P, NCH], f32, tag="lnw")
        lnb_sb = persist.tile([P, NCH], f32, tag="lnb")
        nc.gpsimd.dma_start(out=lnw_sb[:], in_=io["ln_w"].rearrange("(a p) -> p a", p=P))
        nc.gpsimd.dma_start(out=lnb_sb[:], in_=io["ln_b"].rearrange("(a p) -> p a", p=P))
        bv_row = persist.tile([1, C], bf16, tag="bvrow")
        nc.gpsimd.dma_start(out=bv_row[:], in_=io["bv"].rearrange("(a c) -> a c", a=1))
        m_rep = persist.tile([P, L], f32, tag="mrep")
        nc.gpsimd.dma_start(out=m_rep[:], in_=_bcast_ap(io["maskf"], P))

        ones_f = persist.tile([P, 1], f32, tag="onesf")
        nc.vector.memset(ones_f[:], 1.0)
        ones_col = persist.tile([P, 1], f32r, tag="ones")
        nc.vector.tensor_copy(ones_col[:], ones_f[:])
        ones_colb = persist.tile([P, 1], bf16, tag="onesb")
        nc.vector.tensor_copy(ones_colb[:], ones_f[:])
        zero_col = persist.tile([P, 1], f32, tag="zero")
        nc.vector.memset(zero_col[:], 0.0)
        ones_row_f = persist.tile([1, P], f32, tag="onesrowf")
        nc.vector.memset(ones_row_f[:], 1.0)
        ones_row = persist.tile([1, P], f32r, tag="onesrow")
        nc.vector.tensor_copy(ones_row[:], ones_row_f[:])
        ones1 = persist.tile([1, P], bf16, tag="ones1")
        nc.vector.tensor_copy(ones1[:], ones_row_f[:])

        # ---- big persistent tensors --------------------------------------
        seq_hi = persist.tile([P, NCH, L], f8, tag="seqh")
        seq_lo = persist.tile([P, NCH, L], f8, tag="seql")
        w_sb = {}
        for name in ("wq", "wk", "wv"):
            for half in ("hi", "lo"):
                w_sb[name, half] = persist.tile([P, NCH, C], f8,
                                                tag=f"{name}{half}",
                                                name=f"{name}{half}")
        # DMA order = DMA_ENGINES serialization order: critical path first.
        nc.gpsimd.dma_start(
            out=seq_hi[:], in_=io["seq_hi"].rearrange("(cc p) l -> p cc l", p=P))
        nc.gpsimd.dma_start(
            out=w_sb["wq", "hi"][:],
            in_=io["wq_hi"].rearrange("(cc p) t -> p cc t", p=P))
        nc.gpsimd.dma_start(
            out=w_sb["wk", "hi"][:],
            in_=io["wk_hi"].rearrange("(cc p) t -> p cc t", p=P))
        nc.gpsimd.dma_start(
            out=seq_lo[:], in_=io["seq_lo"].rearrange("(cc p) l -> p cc l", p=P))
        nc.gpsimd.dma_start(
            out=w_sb["wq", "lo"][:],
            in_=io["wq_lo"].rearrange("(cc p) t -> p cc t", p=P))
        nc.gpsimd.dma_start(
            out=w_sb["wk", "lo"][:],
            in_=io["wk_lo"].rearrange("(cc p) t -> p cc t", p=P))
        nc.gpsimd.dma_start(
            out=w_sb["wv", "hi"][:],
            in_=io["wv_hi"].rearrange("(cc p) t -> p cc t", p=P))
        nc.gpsimd.dma_start(
            out=w_sb["wv", "lo"][:],
            in_=io["wv_lo"].rearrange("(cc p) t -> p cc t", p=P))

        if USE_FP8_SCORES:
            # [32*(h%4)+d%32, h//4, d//32, pos] per head h, head-channel d
            qT = persist.tile([P, 4, 2, L], f8, tag="qT")
            kT = persist.tile([P, 4, 2, L], f8, tag="kT")
        else:
            qT = persist.tile([P, NCH, L], bf16, tag="qT")
            kT = persist.tile([P, NCH, L], bf16, tag="kT")
        v_aug = persist.tile([P, NLC, H, P], f8, tag="vaug")
        y_sb = persist.tile([P, NCH, L], bf16, tag="y")

        # mask columns 0:64 of v_aug (denominator rows of the AV matmul)
        ml_b = bass.AP(tensor=m_l.tensor, offset=m_l.offset,
                       ap=list(m_l[:].ap[:2]) + [[0, H], [0, DH]])
        nc.gpsimd.tensor_copy(v_aug[:, :, :, 0:DH], ml_b)

        with tc.tile_pool(name="mm", bufs=2, space="PSUM") as mmp, \
             tc.tile_pool(name="scp", bufs=2, space="PSUM") as scp, \
             tc.tile_pool(name="stats", bufs=1, space="PSUM") as statsp, \
             tc.tile_pool(name="eT", bufs=3) as epool, \
             tc.tile_pool(name="att", bufs=2) as attp, \
             tc.tile_pool(name="sq", bufs=2) as sqp:

            # [0:1] = sum_c y, [32:33] = sum_c y*y (accumulated per chunk)
            ps_stats = statsp.tile([33, L], f32, tag="st")

            # ---- emission helpers ----------------------------------------
            PROJ_TERMS = (("hi", "hi"), ("lo", "hi"), ("hi", "lo"))

            def qk_lh(wname, dc, lh):
                """One [128, 512] projection tile of Q or K."""
                tgt = qT if wname == "wq" else kT
                b_sb = bq_sb if wname == "wq" else bk_sb
                xs = {"hi": seq_hi, "lo": seq_lo}
                ps = mmp.tile([P, FD], f32, tag="mm")
                n = len(PROJ_TERMS) * 4
                i = 0
                for wh, xh in PROJ_TERMS:
                    wt = w_sb[wname, wh]
                    xt = xs[xh]
                    for ccp in range(4):
                        nc.tensor.matmul(
                            ps[:],
                            lhsT=wt[:, 2 * ccp:2 * ccp + 2, dc * P:(dc + 1) * P],
                            rhs=xt[:, 2 * ccp:2 * ccp + 2, lh * FD:(lh + 1) * FD],
                            start=(i == 0), stop=(i == n - 1), perf_mode=DR)
                        i += 1
                if USE_FP8_SCORES:
                    dst = tgt[:, dc // 2, dc % 2, lh * FD:(lh + 1) * FD]
                else:
                    dst = tgt[:, dc, lh * FD:(lh + 1) * FD]
                nc.vector.scalar_tensor_tensor(
                    out=dst, in0=ps[:], scalar=b_sb[:, dc:dc + 1],
                    in1=zero_col[:].to_broadcast((P, FD)),
                    op0=ALU.add, op1=ALU.max)

            def projv_lc(dh2, lc):
                """One [128 positions, 512 channels] V tile -> v_aug."""
                xs = {"hi": seq_hi, "lo": seq_lo}
                ps = mmp.tile([P, FD], f32, tag="mm")
                nc.tensor.matmul(
                    ps[:], lhsT=ones1[:], rhs=bv_row[:, dh2 * FD:(dh2 + 1) * FD],
                    start=True, stop=False)
                n = len(PROJ_TERMS) * 4
                i = 0
                for wh, xh in PROJ_TERMS:
                    wt = w_sb["wv", wh]
                    xt = xs[xh]
                    for ccp in range(4):
                        nc.tensor.matmul(
                            ps[:],
                            lhsT=xt[:, 2 * ccp:2 * ccp + 2, lc * P:(lc + 1) * P],
                            rhs=wt[:, 2 * ccp:2 * ccp + 2, dh2 * FD:(dh2 + 1) * FD],
                            start=False, stop=(i == n - 1), perf_mode=DR)
                        i += 1
                # relu + key-position mask on Pool: (ps max 0) * m[l]
                nc.gpsimd.scalar_tensor_tensor(
                    out=v_aug[:, lc, dh2 * (H // 2):(dh2 + 1) * (H // 2), DH:P],
                    in0=ps[:], scalar=0.0,
                    in1=m_l[:, lc:lc + 1].to_broadcast((P, FD)),
                    op0=ALU.max, op1=ALU.mult)

            def av_qh(h, eT, qh):
                """Attention-value matmul + normalization for one query half."""
                ps = mmp.tile([P, FD], f32, tag="mm")
                for kcp in range(4):
                    nc.tensor.matmul(
                        ps[:],
                        lhsT=v_aug[:, 2 * kcp:2 * kcp + 2, h, :],
                        rhs=eT[:, 2 * kcp:2 * kcp + 2, qh * FD:(qh + 1) * FD],
                        start=(kcp == 0), stop=(kcp == 3), perf_mode=DR)
                rcp = attp.tile([DH, FD], f32, tag="rcp")
                nc.vector.reciprocal_approx_fast(out=rcp[:], in_=ps[0:DH, :])
                off = DH * (h % 2)
                nc.gpsimd.tensor_mul(
                    y_sb[off:off + DH, h // 2, qh * FD:(qh + 1) * FD],
                    ps[DH:P, :], rcp[:])

            def stats_cc(cc):
                """Accumulate sum_c y and sum_c y^2 for a finished chunk."""
                ysq = sqp.tile([P, L], f32r, tag="ysq")
                nc.gpsimd.tensor_mul(ysq[:], y_sb[:, cc, :], y_sb[:, cc, :])
                for qh in range(NQ):
                    nc.tensor.matmul(
                        ps_stats[0:1, qh * FD:(qh + 1) * FD],
                        lhsT=ones_colb[:],
                        rhs=y_sb[:, cc, qh * FD:(qh + 1) * FD],
                        start=(cc == 0), stop=(cc == NCH - 1))
                    nc.tensor.matmul(
                        ps_stats[32:33, qh * FD:(qh + 1) * FD],
                        lhsT=ones_col[:], rhs=ysq[:, qh * FD:(qh + 1) * FD],
                        start=(cc == 0), stop=(cc == NCH - 1),
                        tile_position=(0, 32))

            bg = deque()

            def bg_run(n):
                for _ in range(n):
                    if not bg:
                        return
                    bg.popleft()()

            eTs = {}

            def scores_head(h):
                eT = epool.tile([P, NLC, L], f8, tag="eT", name=f"eT{h}")
                eTs[h] = eT
                g, r = h // 4, h % 4
                hp, off = h // 2, DH * (h % 2)
                for kc in range(NLC):
                    ps = scp.tile([P, L], f32, tag="sc")
                    for qh in range(NQ):
                        if USE_FP8_SCORES:
                            nc.tensor.matmul(
                                ps[:, qh * FD:(qh + 1) * FD],
                                lhsT=kT[32 * r:32 * r + 32, g, 0:2,
                                        kc * P:(kc + 1) * P],
                                rhs=qT[32 * r:32 * r + 32, g, 0:2,
                                       qh * FD:(qh + 1) * FD],
                                start=True, stop=True, perf_mode=DR,
                                tile_position=(32 * r, 0))
                        else:
                            nc.tensor.matmul(
                                ps[:, qh * FD:(qh + 1) * FD],
                                lhsT=kT[off:off + DH, hp, kc * P:(kc + 1) * P],
                                rhs=qT[off:off + DH, hp, qh * FD:(qh + 1) * FD],
                                start=True, stop=True)
                    nc.scalar.activation(eT[:, kc, :], ps[:], AF.Exp, scale=SCALE)
                    if kc >= 1:
                        bg_run(1)

            def qk_items(dc):
                return [lambda w=w, dc=dc, lh=lh: qk_lh(w, dc, lh)
                        for w in ("wk", "wq") for lh in range(NQ)]

            def av_items(h):
                return [lambda h=h, qh=qh: av_qh(h, eTs[h], qh)
                        for qh in range(NQ)]

            # ---- prologue: Q/K chunks 0,1 feed the first head group -------
            for dc in (0, 1):
                for it in qk_items(dc):
                    it()

            # background enqueue plan, keyed by head slot
            plan = {
                0: [lambda lc=lc: projv_lc(0, lc) for lc in range(4)],
                1: [lambda lc=lc: projv_lc(0, lc) for lc in range(4, 8)],
                2: av_items(0) + qk_items(2),
                3: av_items(1) + [lambda: stats_cc(0)] + qk_items(3),
                4: av_items(2),
                5: av_items(3) + [lambda: stats_cc(1)] + qk_items(4),
                6: av_items(4) + qk_items(5),
                7: av_items(5) + [lambda: stats_cc(2)]
                   + [lambda lc=lc: projv_lc(1, lc) for lc in range(4)],
                8: av_items(6)
                   + [lambda lc=lc: projv_lc(1, lc) for lc in range(4, 8)],
                9: av_items(7) + [lambda: stats_cc(3)] + qk_items(6),
                10: av_items(8) + qk_items(7),
                11: av_items(9) + [lambda: stats_cc(4)],
                12: av_items(10),
                13: av_items(11) + [lambda: stats_cc(5)],
                14: av_items(12),
                15: av_items(13) + [lambda: stats_cc(6)],
            }
            for h in range(H):
                if h in plan:
                    # av items reference eTs[h'] lazily via av_items closures
                    bg.extend(plan[h])
                scores_head(h)
            while bg:
                bg.popleft()()
            for it in av_items(14) + av_items(15):
                it()
            stats_cc(7)

            # ---- LN row statistics (still inside stats PSUM scope) -------
            rows = persist
            u_row = rows.tile([1, L], f32r, tag="u")
            nc.gpsimd.tensor_scalar_mul(u_row[:], ps_stats[0:1, :], 1.0 / C)
            u2_row = rows.tile([1, L], f32, tag="u2")
            nc.gpsimd.tensor_mul(u2_row[:], u_row[:], u_row[:])
            var_row = rows.tile([1, L], f32, tag="var")
            nc.gpsimd.scalar_tensor_tensor(
                out=var_row[:], in0=ps_stats[32:33, :], scalar=1.0 / C,
                in1=u2_row[:], op0=ALU.mult, op1=ALU.subtract)

        # ---- LayerNorm tail ----------------------------------------------
        rows = persist
        eps_col = rows.tile([1, 1], f32, tag="eps")
        nc.vector.memset(eps_col[:], EPS)
        ln_row = rows.tile([1, L], f32, tag="lnr")
        nc.scalar.activation(ln_row[:], var_row[:], AF.Ln, bias=eps_col[:, 0:1])
        rstd_row = rows.tile([1, L], f32r, tag="rstd")
        nc.scalar.activation(rstd_row[:], ln_row[:], AF.Exp, scale=-0.5)

        with tc.tile_pool(name="tailps", bufs=1, space="PSUM") as tailps, \
             tc.tile_pool(name="norm", bufs=2) as norm:
            u_rep = tailps.tile([P, L], f32, tag="urep")
            rm_rep = tailps.tile([P, L], f32, tag="rmrep")
            for qh in range(NQ):
                nc.tensor.matmul(u_rep[:, qh * FD:(qh + 1) * FD],
                                 lhsT=ones_row[:],
                                 rhs=u_row[:, qh * FD:(qh + 1) * FD],
                                 start=True, stop=True)
                nc.tensor.matmul(rm_rep[:, qh * FD:(qh + 1) * FD],
                                 lhsT=ones_row[:],
                                 rhs=rstd_row[:, qh * FD:(qh + 1) * FD],
                                 start=True, stop=True)

            out_r = out.rearrange("(cc p) l -> p cc l", p=P)
            for cc in range(NCH):
                eng = nc.vector if cc % 2 == 0 else nc.gpsimd
                t1 = norm.tile([P, L], f32, tag="t1", name=f"t1_{cc}")
                eng.tensor_sub(t1[:], y_sb[:, cc, :], u_rep[:])
                t2 = norm.tile([P, L], f32, tag="t2", name=f"t2_{cc}")
                eng.scalar_tensor_tensor(
                    out=t2[:], in0=t1[:], scalar=lnw_sb[:, cc:cc + 1],
                    in1=rm_rep[:], op0=ALU.mult, op1=ALU.mult)
                t3 = norm.tile([P, L], f32, tag="t3", name=f"t3_{cc}")
                eng.scalar_tensor_tensor(
                    out=t3[:], in0=t2[:], scalar=lnb_sb[:, cc:cc + 1],
                    in1=m_rep[:], op0=ALU.add, op1=ALU.mult)
                dmae = nc.sync if cc % 2 == 0 else nc.scalar
                dmae.dma_start(out=out_r[:, cc, :], in_=t3[:])


def _pin_act_table(nc):
    """Make every activation resolve to the one table that contains all the
    functions this kernel uses (Exp, Ln, Copy, Identity), so the compiler
    emits a single LoadActFuncSet."""
    from concourse.hw_specs import get_activation_tables
    keep = "natural_log_exp_and_others"
    try:
        tabs = get_activation_tables(nc.m.arch)
    except Exception:
        return
    if keep not in tabs:
        return
    shared = set(tabs[keep])
    for name, funcs in tabs.items():
        if name != keep:
            funcs -= shared


def build():
    global _BUILT
    if _BUILT is not None:
        return _BUILT
    nc = bacc.Bacc("TRN2", target_bir_lowering=False, debug=False,
                   num_devices=N_CORES)
    _pin_act_table(nc)
    io = {
        "seq_hi": nc.dram_tensor("seq_hi", [C, L], f8, kind="ExternalInput").ap(),
        "seq_lo": nc.dram_tensor("seq_lo", [C, L], f8, kind="ExternalInput").ap(),
        "maskf": nc.dram_tensor("maskf", [L], f32, kind="ExternalInput").ap(),
        "bq": nc.dram_tensor("bq", [C], f32, kind="ExternalInput").ap(),
        "bk": nc.dram_tensor("bk", [C], f32, kind="ExternalInput").ap(),
        "bv": nc.dram_tensor("bv", [C], bf16, kind="ExternalInput").ap(),
        "ln_w": nc.dram_tensor("ln_w", [C], f32, kind="ExternalInput").ap(),
        "ln_b": nc.dram_tensor("ln_b", [C], f32, kind="ExternalInput").ap(),
        "out": nc.dram_tensor("out", [C, L], f32, kind="ExternalOutput").ap(),
    }
    for name in ("wq", "wk", "wv"):
        for half in ("hi", "lo"):
            t = f"{name}_{half}"
            io[t] = nc.dram_tensor(t, [C, C], f8, kind="ExternalInput").ap()
    with tile.TileContext(nc) as tc:
        _emit(tc, io)
    nc.compile()
    _BUILT = nc
    return nc


def _qk_perm():
    """Permutation of W rows so projection PSUM tiles land in the DoubleRow
    score layout: slot (tile tau, partition j) <- channel 64*h + d with
    h = 4*(tau//2) + j//32, d = 32*(tau%2) + j%32."""
    if not USE_FP8_SCORES:
        return np.arange(C)
    perm = np.empty(C, dtype=np.int64)
    for tau in range(NCH):
        for j in range(P):
            h = 4 * (tau // 2) + j // 32
            d = 32 * (tau % 2) + j % 32
            perm[tau * P + j] = 64 * h + d
    return perm


def _split_fp8(a):
    import ml_dtypes
    hi = a.astype(ml_dtypes.float8_e4m3)
    lo = (a - hi.astype(np.float32)).astype(ml_dtypes.float8_e4m3)
    return np.ascontiguousarray(hi), np.ascontiguousarray(lo)


def make_in_maps(seq, mask, wq, bq, wk, bk, wv, bv, ln_w, ln_b):
    import ml_dtypes
    seq = np.asarray(seq, dtype=np.float32)
    mask_f = np.ascontiguousarray(
        np.asarray(mask).astype(np.float32).reshape(N_CORES, L))
    perm = _qk_perm()
    wq_hi, wq_lo = _split_fp8(
        np.asarray(wq, np.float32)[perm, :].T * SW)
    wk_hi, wk_lo = _split_fp8(
        np.asarray(wk, np.float32)[perm, :].T * SW)
    wv_hi, wv_lo = _split_fp8(np.asarray(wv, np.float32).T * SW)
    shared = {
        "wq_hi": wq_hi, "wq_lo": wq_lo,
        "wk_hi": wk_hi, "wk_lo": wk_lo,
        "wv_hi": wv_hi, "wv_lo": wv_lo,
        "bq": np.ascontiguousarray(np.asarray(bq, np.float32)[perm] * SW),
        "bk": np.ascontiguousarray(np.asarray(bk, np.float32)[perm] * SW),
        "bv": np.ascontiguousarray(
            (np.asarray(bv, np.float32) * SW).astype(ml_dtypes.bfloat16)),
        "ln_w": np.ascontiguousarray(np.asarray(ln_w, dtype=np.float32)),
        "ln_b": np.ascontiguousarray(np.asarray(ln_b, dtype=np.float32)),
    }
    maps = []
    for i in range(N_CORES):
        s_hi, s_lo = _split_fp8(seq[i])
        maps.append({"seq_hi": s_hi, "seq_lo": s_lo, "maskf": mask_f[i],
                     **shared})
    return maps


def kernel(seq, mask, wq, bq, wk, bk, wv, bv, ln_w, ln_b):
    global LAST_RESULTS
    nc = build()
    in_maps = make_in_maps(seq, mask, wq, bq, wk, bk, wv, bv, ln_w, ln_b)
    res = run_bass_kernel_spmd(nc, in_maps, list(range(N_CORES)))
    LAST_RESULTS = res
    return np.stack([res.results[i]["out"] for i in range(N_CORES)], axis=0)


# revision 16
# speedup vs baseline: 1.3288x; 1.0847x over previous
"""nn_AttentionOpt on 8 Trainium2 NeuronCores.

Data-parallel over batch N=8: one batch element per core. Per core
(C=1024 channels, L=1024 positions, H=16 heads, dh=64):

    x = seq^T                        (L, C)
    Q/K = relu(x @ Wq^T + b)         fp8e4, score-sharded layout (see below)
    V   = relu(x @ Wv^T + bv) * m    fp8e4, augmented with the key mask as
                                     64 extra rows (softmax denominator)
    S^T = K_h Q_h^T                  fp8 DoubleRow matmuls ([k, q] layout)
    E   = exp(S^T / (8*SW^2))        ACT, written as fp8e4
    P   = [m*ones | V_h]^T E^T       fp8 DoubleRow: rows 0:64 = denominator,
                                     rows 64:128 = unnormalized out^T
    y_h = P[64:128] * approx_recip(P[0:64])      (f32, kept for LN stats)
    LayerNorm over C via one-pass stats (ones-matmul partition reduce of
    y and y*y during attention), post-mask only.

Performance structure (cost-model driven):
  * All projections run as fp8e4 DoubleRow matmuls (0.5 cycles/row, double
    contraction) with an error-compensating split: x = x_hi + x_lo and
    W = W_hi + W_lo in fp8, computing hi*hi + lo*hi + hi*lo. This gives
    ~bf16-level accuracy at ~2.7x the bf16 matmul rate.
  * Weights are staged host-side: pre-transposed, pre-scaled by SW=32 (to
    keep fp8 values in the normal range), pre-split hi/lo, and (for Wq/Wk)
    row-permuted so the projection PSUM tiles land directly in the
    [32-channel, 2-ksubtile] layout DoubleRow score matmuls need.
  * Scores use single fp8 Q/K (quantization error is suppressed by softmax
    averaging over ~512 unmasked keys); AV uses single fp8 E and V.
  * The exp stream on the ACT engine (~128 x [128,1024] activations) is the
    bottleneck; PE work is interleaved behind it via a background queue.

Scale bookkeeping: Q,K,V all carry SW=32 -> scores carry SW^2 (folded into
the exp scale), y carries SW (cancelled by LayerNorm; EPS scaled by SW^2).
"""
import sys

if "/opt/trn_rl_repo" not in sys.path:
    sys.path.insert(0, "/opt/trn_rl_repo")

from collections import deque
from contextlib import ExitStack

import numpy as np

import concourse.bass as bass
import concourse.tile as tile
from concourse import bacc, mybir
from concourse.bass_utils import run_bass_kernel_spmd

f32 = mybir.dt.float32
f32r = mybir.dt.float32r
bf16 = mybir.dt.bfloat16
f8 = mybir.dt.float8e4
AF = mybir.ActivationFunctionType
ALU = mybir.AluOpType
DR = mybir.MatmulPerfMode.DoubleRow

N_CORES = 8
C = 1024
L = 1024
H = 16
DH = 64
P = 128
NCH = C // P          # 8 chunks of channels
NLC = L // P          # 8 chunks of positions (key chunks)
FD = 512              # matmul moving free dim (one PSUM bank of f32)
NQ = L // FD          # 2 query halves
SW = 32.0             # host-side weight scale (fp8 normal range)
EPS = 1e-5 * SW * SW  # LN epsilon in the SW-scaled domain
SCALE = 1.0 / (8.0 * SW * SW)   # 1/sqrt(dh) folded with 1/SW^2

USE_FP8_SCORES = True

_BUILT = None
LAST_RESULTS = None


def _bcast_ap(ap, n):
    """Prepend a [0, n] partition-broadcast dim to an AP (DRAM source)."""
    return bass.AP(tensor=ap.tensor, offset=ap.offset, ap=[[0, n]] + list(ap.ap))


def _emit(tc, io):
    nc = tc.nc
    out = io["out"]

    with ExitStack() as ctx:
        persist = ctx.enter_context(tc.tile_pool(name="persist", bufs=1))

        # ---- constants / small inputs ------------------------------------
        bq_sb = persist.tile([P, NCH], f32, tag="bq")
        bk_sb = persist.tile([P, NCH], f32, tag="bk")
        nc.gpsimd.dma_start(out=bq_sb[:], in_=io["bq"].rearrange("(a p) -> p a", p=P))
        nc.gpsimd.dma_start(out=bk_sb[:], in_=io["bk"].rearrange("(a p) -> p a", p=P))
        m_l = persist.tile([P, NLC], f32, tag="ml")
        nc.gpsimd.dma_start(out=m_l[:], in_=io["maskf"].rearrange("(a p) -> p a", p=P))
        lnw_sb = persist.tile([P, NCH], f32, tag="lnw")
        lnb_sb = persist.tile([P, NCH], f32, tag="lnb")
        nc.gpsimd.dma_start(out=lnw_sb[:], in_=io["ln_w"].rearrange("(a p) -> p a", p=P))
        nc.gpsimd.dma_start(out=lnb_sb[:], in_=io["ln_b"].rearrange("(a p) -> p a", p=P))
        bv_row = persist.tile([1, C], bf16, tag="bvrow")
        nc.gpsimd.dma_start(out=bv_row[:], in_=io["bv"].rearrange("(a c) -> a c", a=1))
        m_rep = persist.tile([P, L], f32, tag="mrep")
        nc.gpsimd.dma_start(out=m_rep[:], in_=_bcast_ap(io["maskf"], P))

        ones_f = persist.tile([P, 1], f32, tag="onesf")
        nc.vector.memset(ones_f[:], 1.0)
        ones_col = persist.tile([P, 1], f32r, tag="ones")
        nc.vector.tensor_copy(ones_col[:], ones_f[:])
        ones_colb = persist.tile([P, 1], bf16, tag="onesb")
        nc.vector.tensor_copy(ones_colb[:], ones_f[:])
        zero_col = persist.tile([P, 1], f32, tag="zero")
        nc.vector.memset(zero_col[:], 0.0)
        ones_row_f = persist.tile([1, P], f32, tag="onesrowf")
        nc.vector.memset(ones_row_f[:], 1.0)
        ones_row = persist.tile([1, P], f32r, tag="onesrow")
        nc.vector.tensor_copy(ones_row[:], ones_row_f[:])
        ones1 = persist.tile([1, P], bf16, tag="ones1")
        nc.vector.tensor_copy(ones1[:], ones_row_f[:])

        # ---- big persistent tensors --------------------------------------
        seq_hi = persist.tile([P, NCH, L], f8, tag="seqh")
        seq_lo = persist.tile([P, NCH, L], f8, tag="seql")
        w_sb = {}
        for name in ("wq", "wk", "wv"):
            for half in ("hi", "lo"):
                w_sb[name, half] = persist.tile([P, NCH, C], f8,
                                                tag=f"{name}{half}",
                                                name=f"{name}{half}")
        # DMA order = DMA_ENGINES serialization order: critical path first.
        # HWDGE queues (sync/vector/scalar) keep descriptor gen off Pool.
        nc.sync.dma_start(
            out=seq_hi[:], in_=io["seq_hi"].rearrange("(cc p) l -> p cc l", p=P))
        nc.sync.dma_start(
            out=w_sb["wq", "hi"][:],
            in_=io["wq_hi"].rearrange("(cc p) t -> p cc t", p=P))
        nc.sync.dma_start(
            out=w_sb["wk", "hi"][:],
            in_=io["wk_hi"].rearrange("(cc p) t -> p cc t", p=P))
        nc.scalar.dma_start(
            out=seq_lo[:], in_=io["seq_lo"].rearrange("(cc p) l -> p cc l", p=P))
        nc.scalar.dma_start(
            out=w_sb["wq", "lo"][:],
            in_=io["wq_lo"].rearrange("(cc p) t -> p cc t", p=P))
        nc.scalar.dma_start(
            out=w_sb["wk", "lo"][:],
            in_=io["wk_lo"].rearrange("(cc p) t -> p cc t", p=P))
        nc.sync.dma_start(
            out=w_sb["wv", "hi"][:],
            in_=io["wv_hi"].rearrange("(cc p) t -> p cc t", p=P))
        nc.sync.dma_start(
            out=w_sb["wv", "lo"][:],
            in_=io["wv_lo"].rearrange("(cc p) t -> p cc t", p=P))

        if USE_FP8_SCORES:
            # [32*(h%4)+d%32, h//4, d//32, pos] per head h, head-channel d
            qT = persist.tile([P, 4, 2, L], f8, tag="qT")
            kT = persist.tile([P, 4, 2, L], f8, tag="kT")
        else:
            qT = persist.tile([P, NCH, L], bf16, tag="qT")
            kT = persist.tile([P, NCH, L], bf16, tag="kT")
        v_aug = persist.tile([P, NLC, H, P], f8, tag="vaug")
        y_sb = persist.tile([P, NCH, L], bf16, tag="y")

        # mask columns 0:64 of v_aug (denominator rows of the AV matmul)
        ml_b = bass.AP(tensor=m_l.tensor, offset=m_l.offset,
                       ap=list(m_l[:].ap[:2]) + [[0, H], [0, DH]])
        nc.gpsimd.tensor_copy(v_aug[:, :, :, 0:DH], ml_b)

        with tc.tile_pool(name="mm", bufs=2, space="PSUM") as mmp, \
             tc.tile_pool(name="scp", bufs=2, space="PSUM") as scp, \
             tc.tile_pool(name="stats", bufs=1, space="PSUM") as statsp, \
             tc.tile_pool(name="eT", bufs=3) as epool, \
             tc.tile_pool(name="att", bufs=2) as attp, \
             tc.tile_pool(name="sq", bufs=2) as sqp:

            # [0:1] = sum_c y, [32:33] = sum_c y*y (accumulated per chunk)
            ps_stats = statsp.tile([33, L], f32, tag="st")

            # ---- emission helpers ----------------------------------------
            PROJ_TERMS = (("hi", "hi"), ("lo", "hi"), ("hi", "lo"))

            def qk_lh(wname, dc, lh):
                """One [128, 512] projection tile of Q or K."""
                tgt = qT if wname == "wq" else kT
                b_sb = bq_sb if wname == "wq" else bk_sb
                xs = {"hi": seq_hi, "lo": seq_lo}
                ps = mmp.tile([P, FD], f32, tag="mm")
                n = len(PROJ_TERMS) * 4
                i = 0
                for wh, xh in PROJ_TERMS:
                    wt = w_sb[wname, wh]
                    xt = xs[xh]
                    for ccp in range(4):
                        nc.tensor.matmul(
                            ps[:],
                            lhsT=wt[:, 2 * ccp:2 * ccp + 2, dc * P:(dc + 1) * P],
                            rhs=xt[:, 2 * ccp:2 * ccp + 2, lh * FD:(lh + 1) * FD],
                            start=(i == 0), stop=(i == n - 1), perf_mode=DR)
                        i += 1
                if USE_FP8_SCORES:
                    dst = tgt[:, dc // 2, dc % 2, lh * FD:(lh + 1) * FD]
                else:
                    dst = tgt[:, dc, lh * FD:(lh + 1) * FD]
                nc.vector.scalar_tensor_tensor(
                    out=dst, in0=ps[:], scalar=b_sb[:, dc:dc + 1],
                    in1=zero_col[:].to_broadcast((P, FD)),
                    op0=ALU.add, op1=ALU.max)

            def projv_lc(dh2, lc):
                """One [128 positions, 512 channels] V tile -> v_aug."""
                xs = {"hi": seq_hi, "lo": seq_lo}
                ps = mmp.tile([P, FD], f32, tag="mm")
                nc.tensor.matmul(
                    ps[:], lhsT=ones1[:], rhs=bv_row[:, dh2 * FD:(dh2 + 1) * FD],
                    start=True, stop=False)
                n = len(PROJ_TERMS) * 4
                i = 0
                for wh, xh in PROJ_TERMS:
                    wt = w_sb["wv", wh]
                    xt = xs[xh]
                    for ccp in range(4):
                        nc.tensor.matmul(
                            ps[:],
                            lhsT=xt[:, 2 * ccp:2 * ccp + 2, lc * P:(lc + 1) * P],
                            rhs=wt[:, 2 * ccp:2 * ccp + 2, dh2 * FD:(dh2 + 1) * FD],
                            start=False, stop=(i == n - 1), perf_mode=DR)
                        i += 1
                # relu + key-position mask: (ps max 0) * m[l]  (DVE: PSUM in)
                nc.vector.scalar_tensor_tensor(
                    out=v_aug[:, lc, dh2 * (H // 2):(dh2 + 1) * (H // 2), DH:P],
                    in0=ps[:], scalar=0.0,
                    in1=m_l[:, lc:lc + 1].to_broadcast((P, FD)),
                    op0=ALU.max, op1=ALU.mult)

            def av_qh(h, eT, qh):
                """Attention-value matmul + normalization for one query half."""
                ps = mmp.tile([P, FD], f32, tag="mm")
                for kcp in range(4):
                    nc.tensor.matmul(
                        ps[:],
                        lhsT=v_aug[:, 2 * kcp:2 * kcp + 2, h, :],
                        rhs=eT[:, 2 * kcp:2 * kcp + 2, qh * FD:(qh + 1) * FD],
                        start=(kcp == 0), stop=(kcp == 3), perf_mode=DR)
                rcp = attp.tile([DH, FD], f32, tag="rcp")
                nc.vector.reciprocal_approx_fast(out=rcp[:], in_=ps[0:DH, :])
                off = DH * (h % 2)
                nc.vector.tensor_mul(
                    y_sb[off:off + DH, h // 2, qh * FD:(qh + 1) * FD],
                    ps[DH:P, :], rcp[:])

            def stats_cc(cc):
                """Accumulate sum_c y and sum_c y^2 for a finished chunk."""
                ysq = sqp.tile([P, L], f32r, tag="ysq")
                nc.gpsimd.tensor_mul(ysq[:], y_sb[:, cc, :], y_sb[:, cc, :])
                for qh in range(NQ):
                    nc.tensor.matmul(
                        ps_stats[0:1, qh * FD:(qh + 1) * FD],
                        lhsT=ones_colb[:],
                        rhs=y_sb[:, cc, qh * FD:(qh + 1) * FD],
                        start=(cc == 0), stop=(cc == NCH - 1))
                    nc.tensor.matmul(
                        ps_stats[32:33, qh * FD:(qh + 1) * FD],
                        lhsT=ones_col[:], rhs=ysq[:, qh * FD:(qh + 1) * FD],
                        start=(cc == 0), stop=(cc == NCH - 1),
                        tile_position=(0, 32))

            bg = deque()

            def bg_run(n):
                for _ in range(n):
                    if not bg:
                        return
                    bg.popleft()()

            eTs = {}

            def scores_head(h):
                eT = epool.tile([P, NLC, L], f8, tag="eT", name=f"eT{h}")
                eTs[h] = eT
                g, r = h // 4, h % 4
                hp, off = h // 2, DH * (h % 2)
                for kc in range(NLC):
                    ps = scp.tile([P, L], f32, tag="sc")
                    for qh in range(NQ):
                        if USE_FP8_SCORES:
                            nc.tensor.matmul(
                                ps[:, qh * FD:(qh + 1) * FD],
                                lhsT=kT[32 * r:32 * r + 32, g, 0:2,
                                        kc * P:(kc + 1) * P],
                                rhs=qT[32 * r:32 * r + 32, g, 0:2,
                                       qh * FD:(qh + 1) * FD],
                                start=True, stop=True, perf_mode=DR,
                                tile_position=(32 * r, 0))
                        else:
                            nc.tensor.matmul(
                                ps[:, qh * FD:(qh + 1) * FD],
                                lhsT=kT[off:off + DH, hp, kc * P:(kc + 1) * P],
                                rhs=qT[off:off + DH, hp, qh * FD:(qh + 1) * FD],
                                start=True, stop=True)
                    nc.scalar.activation(eT[:, kc, :], ps[:], AF.Exp, scale=SCALE)
                    if kc >= 1:
                        bg_run(1)

            def qk_items(dc):
                return [lambda w=w, dc=dc, lh=lh: qk_lh(w, dc, lh)
                        for w in ("wk", "wq") for lh in range(NQ)]

            def av_items(h):
                return [lambda h=h, qh=qh: av_qh(h, eTs[h], qh)
                        for qh in range(NQ)]

            # ---- prologue: Q/K chunks 0,1 feed the first head group -------
            for dc in (0, 1):
                for it in qk_items(dc):
                    it()

            # background enqueue plan, keyed by head slot
            plan = {
                0: [lambda lc=lc: projv_lc(0, lc) for lc in range(4)],
                1: [lambda lc=lc: projv_lc(0, lc) for lc in range(4, 8)],
                2: av_items(0) + qk_items(2),
                3: av_items(1) + [lambda: stats_cc(0)] + qk_items(3),
                4: av_items(2),
                5: av_items(3) + [lambda: stats_cc(1)] + qk_items(4),
                6: av_items(4) + qk_items(5),
                7: av_items(5) + [lambda: stats_cc(2)]
                   + [lambda lc=lc: projv_lc(1, lc) for lc in range(4)],
                8: av_items(6)
                   + [lambda lc=lc: projv_lc(1, lc) for lc in range(4, 8)],
                9: av_items(7) + [lambda: stats_cc(3)] + qk_items(6),
                10: av_items(8) + qk_items(7),
                11: av_items(9) + [lambda: stats_cc(4)],
                12: av_items(10),
                13: av_items(11) + [lambda: stats_cc(5)],
                14: av_items(12),
                15: av_items(13) + [lambda: stats_cc(6)],
            }
            for h in range(H):
                if h in plan:
                    # av items reference eTs[h'] lazily via av_items closures
                    bg.extend(plan[h])
                scores_head(h)
            while bg:
                bg.popleft()()
            for it in av_items(14) + av_items(15):
                it()
            stats_cc(7)

            # ---- LN row statistics (still inside stats PSUM scope) -------
            rows = persist
            u_row = rows.tile([1, L], f32r, tag="u")
            nc.vector.tensor_scalar_mul(u_row[:], ps_stats[0:1, :], 1.0 / C)
            u2_row = rows.tile([1, L], f32, tag="u2")
            nc.gpsimd.tensor_mul(u2_row[:], u_row[:], u_row[:])
            var_row = rows.tile([1, L], f32, tag="var")
            nc.vector.scalar_tensor_tensor(
                out=var_row[:], in0=ps_stats[32:33, :], scalar=1.0 / C,
                in1=u2_row[:], op0=ALU.mult, op1=ALU.subtract)

        # ---- LayerNorm tail ----------------------------------------------
        rows = persist
        eps_col = rows.tile([1, 1], f32, tag="eps")
        nc.vector.memset(eps_col[:], EPS)
        ln_row = rows.tile([1, L], f32, tag="lnr")
        nc.scalar.activation(ln_row[:], var_row[:], AF.Ln, bias=eps_col[:, 0:1])
        rstd_row = rows.tile([1, L], f32r, tag="rstd")
        nc.scalar.activation(rstd_row[:], ln_row[:], AF.Exp, scale=-0.5)

        with tc.tile_pool(name="tailps", bufs=1, space="PSUM") as tailps, \
             tc.tile_pool(name="norm", bufs=2) as norm:
            u_rep = tailps.tile([P, L], f32, tag="urep")
            rm_rep = tailps.tile([P, L], f32, tag="rmrep")
            for qh in range(NQ):
                nc.tensor.matmul(u_rep[:, qh * FD:(qh + 1) * FD],
                                 lhsT=ones_row[:],
                                 rhs=u_row[:, qh * FD:(qh + 1) * FD],
                                 start=True, stop=True)
                nc.tensor.matmul(rm_rep[:, qh * FD:(qh + 1) * FD],
                                 lhsT=ones_row[:],
                                 rhs=rstd_row[:, qh * FD:(qh + 1) * FD],
                                 start=True, stop=True)

            # Pool (gpsimd) cannot read PSUM: stage the broadcasts in SBUF
            u_rep_sb = norm.tile([P, L], f32, tag="ureps", bufs=1)
            nc.vector.tensor_copy(u_rep_sb[:], u_rep[:])
            rm_rep_sb = norm.tile([P, L], f32, tag="rmreps", bufs=1)
            nc.vector.tensor_copy(rm_rep_sb[:], rm_rep[:])

            out_r = out.rearrange("(cc p) l -> p cc l", p=P)
            for cc in range(NCH):
                eng = nc.vector if cc % 2 == 0 else nc.gpsimd
                t1 = norm.tile([P, L], f32, tag="t1", name=f"t1_{cc}")
                eng.tensor_sub(t1[:], y_sb[:, cc, :], u_rep_sb[:])
                t2 = norm.tile([P, L], f32, tag="t2", name=f"t2_{cc}")
                eng.scalar_tensor_tensor(
                    out=t2[:], in0=t1[:], scalar=lnw_sb[:, cc:cc + 1],
                    in1=rm_rep_sb[:], op0=ALU.mult, op1=ALU.mult)
                t3 = norm.tile([P, L], f32, tag="t3", name=f"t3_{cc}")
                eng.scalar_tensor_tensor(
                    out=t3[:], in0=t2[:], scalar=lnb_sb[:, cc:cc + 1],
                    in1=m_rep[:], op0=ALU.add, op1=ALU.mult)
                dmae = nc.sync if cc % 2 == 0 else nc.scalar
                dmae.dma_start(out=out_r[:, cc, :], in_=t3[:])


def _pin_act_table(nc):
    """Make every activation resolve to the one table that contains all the
    functions this kernel uses (Exp, Ln, Copy, Identity), so the compiler
    emits a single LoadActFuncSet."""
    from concourse.hw_specs import get_activation_tables
    keep = "natural_log_exp_and_others"
    try:
        tabs = get_activation_tables(nc.m.arch)
    except Exception:
        return
    if keep not in tabs:
        return
    shared = set(tabs[keep])
    for name, funcs in tabs.items():
        if name != keep:
            funcs -= shared


def build():
    global _BUILT
    if _BUILT is not None:
        return _BUILT
    nc = bacc.Bacc("TRN2", target_bir_lowering=False, debug=False,
                   num_devices=N_CORES)
    _pin_act_table(nc)
    io = {
        "seq_hi": nc.dram_tensor("seq_hi", [C, L], f8, kind="ExternalInput").ap(),
        "seq_lo": nc.dram_tensor("seq_lo", [C, L], f8, kind="ExternalInput").ap(),
        "maskf": nc.dram_tensor("maskf", [L], f32, kind="ExternalInput").ap(),
        "bq": nc.dram_tensor("bq", [C], f32, kind="ExternalInput").ap(),
        "bk": nc.dram_tensor("bk", [C], f32, kind="ExternalInput").ap(),
        "bv": nc.dram_tensor("bv", [C], bf16, kind="ExternalInput").ap(),
        "ln_w": nc.dram_tensor("ln_w", [C], f32, kind="ExternalInput").ap(),
        "ln_b": nc.dram_tensor("ln_b", [C], f32, kind="ExternalInput").ap(),
        "out": nc.dram_tensor("out", [C, L], f32, kind="ExternalOutput").ap(),
    }
    for name in ("wq", "wk", "wv"):
        for half in ("hi", "lo"):
            t = f"{name}_{half}"
            io[t] = nc.dram_tensor(t, [C, C], f8, kind="ExternalInput").ap()
    with tile.TileContext(nc) as tc:
        _emit(tc, io)
    nc.compile()
    _BUILT = nc
    return nc


def _qk_perm():
    """Permutation of W rows so projection PSUM tiles land in the DoubleRow
    score layout: slot (tile tau, partition j) <- channel 64*h + d with
    h = 4*(tau//2) + j//32, d = 32*(tau%2) + j%32."""
    if not USE_FP8_SCORES:
        return np.arange(C)
    perm = np.empty(C, dtype=np.int64)
    for tau in range(NCH):
        for j in range(P):
            h = 4 * (tau // 2) + j // 32
            d = 32 * (tau % 2) + j % 32
            perm[tau * P + j] = 64 * h + d
    return perm


def _split_fp8(a):
    import ml_dtypes
    hi = a.astype(ml_dtypes.float8_e4m3)
    lo = (a - hi.astype(np.float32)).astype(ml_dtypes.float8_e4m3)
    return np.ascontiguousarray(hi), np.ascontiguousarray(lo)


def make_in_maps(seq, mask, wq, bq, wk, bk, wv, bv, ln_w, ln_b):
    import ml_dtypes
    seq = np.asarray(seq, dtype=np.float32)
    mask_f = np.ascontiguousarray(
        np.asarray(mask).astype(np.float32).reshape(N_CORES, L))
    perm = _qk_perm()
    wq_hi, wq_lo = _split_fp8(
        np.asarray(wq, np.float32)[perm, :].T * SW)
    wk_hi, wk_lo = _split_fp8(
        np.asarray(wk, np.float32)[perm, :].T * SW)
    wv_hi, wv_lo = _split_fp8(np.asarray(wv, np.float32).T * SW)
    shared = {
        "wq_hi": wq_hi, "wq_lo": wq_lo,
        "wk_hi": wk_hi, "wk_lo": wk_lo,
        "wv_hi": wv_hi, "wv_lo": wv_lo,
        "bq": np.ascontiguousarray(np.asarray(bq, np.float32)[perm] * SW),
        "bk": np.ascontiguousarray(np.asarray(bk, np.float32)[perm] * SW),
        "bv": np.ascontiguousarray(
            (np.asarray(bv, np.float32) * SW).astype(ml_dtypes.bfloat16)),
        "ln_w": np.ascontiguousarray(np.asarray(ln_w, dtype=np.float32)),
        "ln_b": np.ascontiguousarray(np.asarray(ln_b, dtype=np.float32)),
    }
    maps = []
    for i in range(N_CORES):
        s_hi, s_lo = _split_fp8(seq[i])
        maps.append({"seq_hi": s_hi, "seq_lo": s_lo, "maskf": mask_f[i],
                     **shared})
    return maps


def kernel(seq, mask, wq, bq, wk, bk, wv, bv, ln_w, ln_b):
    global LAST_RESULTS
    nc = build()
    in_maps = make_in_maps(seq, mask, wq, bq, wk, bk, wv, bv, ln_w, ln_b)
    res = run_bass_kernel_spmd(nc, in_maps, list(range(N_CORES)))
    LAST_RESULTS = res
    return np.stack([res.results[i]["out"] for i in range(N_CORES)], axis=0)
